# revision 36
# baseline (speedup 1.0000x reference)
"""Trainium2 Bass kernel for nn_ExhustiveContrastiveLoss (v2).

Reference computation (N=8192, D=512, fp32):
    xd = normalize(embed_data); xl = normalize(embed_label)
    f2f = xd @ xd.T with diagonal removed; e2p = xd @ xl.T (full)
    per-strip row max subtracted before exp (the two strips use DIFFERENT
    maxes inside the same num/den sums, so the maxes are load-bearing)
    num = sum(pos * e2p_logits) + sum(pos_nd * f2f_logits)
    den = sum(e2p_logits) + sum(f2f_logits)
    loss = -mean(log(num / den))

v2 changes over the fp32r baseline:
  * Rows AND columns are permuted into label-sorted order on the host (the
    loss is a mean over rows, so row order is free; column order only
    permutes sums). All positive pairs (label_i == label_j) then live in a
    narrow diagonal band, so the masked numerator sum only needs a static
    512-column window per 128-row tile instead of the full row: the DVE
    scalar_tensor_tensor drops from 8192 to 512 columns per row.
  * Matmuls run in fp8 (e4m3, inputs scaled by 16) with
    perf_mode=DoubleRow (2 fp8 weights per PE cell, K=256 per instr).
  * Loop is row-tile-outer so each (row tile, strip) produces one
    [128, 4096] bf16 logits tile for a single max tree + window STT.

Sharding: 4x2 grid over 8 cores. Core k: row shard r = k % 4 (2048 sorted
rows), col shard c = k // 4. Core (r, 0) takes sorted cols
[2048r-512, 2048r+3584) mod 8192 (covers the positive band of its rows);
core (r, 1) takes the complementary 4096 cols. Per row it computes
    C' = sum_j exp((S - 1)/T)        (shift-1 denominator partial)
    A' = sum_j pos_ij exp((S - 1)/T) (numerator partial; 0 on c=1 cores
                                      via a sentinel column-label tensor)
    M  = max_j exp((S - 1)/T)        (shard row max of the logits)
The host combines shards exactly as the reference would (see _combine).

The f2f diagonal is removed by accumulating 16 * (-120) = -1920 into the
diagonal PSUM cells via one identity-weight fp8 matmul per row tile whose
rhs is a per-core host input (all zeros on c=1 cores), keeping the SPMD
program core-uniform. exp under the -1920 shift underflows to exactly 0.
"""

import os

os.environ.setdefault("MYCRO_LOCAL_CACHE", "1")

import numpy as np

import concourse.bass as bass
import concourse.bacc as bacc
import concourse.tile as tile
from concourse import mybir
from concourse.bass_utils import run_bass_kernel_spmd

# Problem constants (hardcoded per harness contract).
N, D = 8192, 512
NCORES = 8
RGRID, CGRID = 4, 2          # 4 row shards x 2 col shards
R = N // RGRID               # 2048 rows per core
C = N // CGRID               # 4096 cols per core
NIT = R // 128               # 16 row tiles
KS = D // 128                # 4 k-subtiles
TEMP = 0.07
EPS = 1e-8
SHIFT = 1.0                  # fixed exp shift; cos sim <= 1
FSCALE = 16.0                # fp8 input scale; psum = 256 * cos
NEGM = -120.0                # mask rhs value (fp8-exact)
EYEV = 16.0                  # identity weight value (fp8-exact)
COLSH = 512                  # col-shard left extension before row diag
WOFF = 448                   # window start offset: W(it)=[128it+448,+256)
WLEN = 256

F32 = mybir.dt.float32
F8 = mybir.dt.float8e4
BF16 = mybir.dt.bfloat16
AX = mybir.AxisListType
OP = mybir.AluOpType
AF = mybir.ActivationFunctionType
DR = mybir.MatmulPerfMode.DoubleRow


def _label_encode(lab):
    """Map class ids 0..999 to distinct bf16-exact floats."""
    lab = np.asarray(lab).astype(np.int64)
    return ((128 + (lab % 128)) * (2.0 ** (lab // 128))).astype(np.float32)


def build_nc():
    nc = bacc.Bacc(
        "TRN2",
        target_bir_lowering=False,
        debug=False,
        num_devices=NCORES,
    )

    # lhs/rhs* hold k-subtile blocks back to back: [128, ksub, cols]
    lhs = nc.declare_dram_parameter("lhs", [128, KS * R], F8, isOutput=False)
    rhsD = nc.declare_dram_parameter("rhsD", [128, KS * C], F8, isOutput=False)
    rhsL = nc.declare_dram_parameter("rhsL", [128, KS * C], F8, isOutput=False)
    labs = nc.declare_dram_parameter("labs", [128, NIT], F32, isOutput=False)
    labc = nc.declare_dram_parameter("labc", [128, C], BF16, isOutput=False)
    mA = nc.declare_dram_parameter("mA", [128, 896], F8, isOutput=False)
    eyeK = nc.declare_dram_parameter("eyeK", [128, 128], F8, isOutput=False)

    # single output: 6 groups of NIT columns: df, mf, nf, de, me, ne
    ofe = nc.declare_dram_parameter("ofe", [128, 6 * NIT], F32, isOutput=True)

    with tile.TileContext(nc) as tc:
        with (
            tc.tile_pool(name="const", bufs=1) as const,
            tc.tile_pool(name="psum", bufs=2, space="PSUM") as psum,
            tc.tile_pool(name="lp", bufs=3) as lp,
            tc.tile_pool(name="mtp", bufs=2) as mtp,
            tc.tile_pool(name="mlp", bufs=2) as mlp,
            tc.tile_pool(name="statp", bufs=1) as statp,
        ):
            dma = nc.default_dma_engine

            lhs_sb = const.tile([128, KS, R], F8, tag="lhs", name="lhs_sb")
            rhs_sb = {
                "f": const.tile([128, KS, C], F8, tag="rhsf", name="rhsf_sb"),
                "e": const.tile([128, KS, C], F8, tag="rhse", name="rhse_sb"),
            }
            # two DMA queues, few big transfers, critical-path order:
            # sync: consts + lhs + col-half 0 of both rhs + labc window;
            # scalar: col-half 1 of both rhs (issued at t=0, lands by the
            # time the first h1 matmuls need it)
            qa, qb = nc.sync, nc.scalar
            rhsDv = rhsD.rearrange("p (k c) -> p k c", k=KS)
            rhsLv = rhsL.rearrange("p (k c) -> p k c", k=KS)
            eye_sb = const.tile([128, 128], F8, tag="eyeK")
            mA_sb = const.tile([128, 896], F8, tag="mA")
            labs_sb = const.tile([128, NIT], F32, tag="labs")
            labc_sb = const.tile([128, C], BF16, tag="labc")
            wlo, whi = WOFF, 128 * (NIT - 1) + WOFF + WLEN
            # k-pair-sized chunks balanced over both queues, in exactly
            # the order the compute consumes them; small consts after the
            # first matmuls' operands (they are needed a few us later)
            lhsv = lhs.rearrange("p (k c) -> p k c", k=KS)
            qa.dma_start(out=lhs_sb[:, 0:2, 0:256], in_=lhsv[:, 0:2, 0:256])
            qb.dma_start(out=lhs_sb[:, 2:4, 0:256], in_=lhsv[:, 2:4, 0:256])
            qa.dma_start(
                out=rhs_sb["f"][:, 0:2, 0:2048], in_=rhsDv[:, 0:2, 0:2048]
            )
            qb.dma_start(
                out=rhs_sb["f"][:, 2:4, 0:2048], in_=rhsDv[:, 2:4, 0:2048]
            )
            qa.dma_start(out=eye_sb, in_=eyeK[:, :])
            qb.dma_start(out=mA_sb, in_=mA[:, :])
            qa.dma_start(out=labs_sb, in_=labs[:, :])
            qa.dma_start(out=lhs_sb[:, 0:2, 256:R], in_=lhsv[:, 0:2, 256:R])
            qb.dma_start(out=lhs_sb[:, 2:4, 256:R], in_=lhsv[:, 2:4, 256:R])
            for mname, dram, half in (
                ("f", rhsDv, 1), ("e", rhsLv, 0), ("e", rhsLv, 1)
            ):
                hs = slice(half * 2048, (half + 1) * 2048)
                qa.dma_start(
                    out=rhs_sb[mname][:, 0:2, hs], in_=dram[:, 0:2, hs]
                )
                qb.dma_start(
                    out=rhs_sb[mname][:, 2:4, hs], in_=dram[:, 2:4, hs]
                )
                if mname == "f":
                    qb.dma_start(
                        out=labc_sb[:, wlo:whi], in_=labc[:, wlo:whi]
                    )
            bias_sb = const.tile([128, 1], F32, tag="expbias")
            nc.vector.memset(bias_sb, -SHIFT / TEMP)

            # one stats tile: [0:64) den-half scratch (f then e), [64:160)
            # final 6xNIT block (df, mf, nf, de, me, ne) shipped as one DMA
            stall = statp.tile([128, 160], F32, tag="stall", name="stall")
            DH = {"f": 0, "e": 32}
            FD = {"f": 64, "e": 112}
            FM = {"f": 80, "e": 128}
            FN = {"f": 96, "e": 144}
            ofe_v = ofe.rearrange("p (g t) -> p g t", t=NIT)
            stf_v = stall[:, 64:160].rearrange("p (g t) -> p g t", t=NIT)

            exp_scale = 1.0 / (FSCALE * FSCALE * TEMP)

            def ship_stats(lo, hi):
                # reduce den halves and DMA finished stat slots [lo, hi)
                for sname in ("f", "e"):
                    nc.vector.tensor_reduce(
                        out=stall[:, FD[sname] + lo:FD[sname] + hi],
                        in_=stall[
                            :, DH[sname] + 2 * lo:DH[sname] + 2 * hi
                        ].rearrange("p (a b) -> p a b", b=2),
                        axis=AX.X,
                        op=OP.add,
                    )
                dma.dma_start(
                    out=ofe_v[:, :, lo:hi], in_=stf_v[:, :, lo:hi]
                )

            for it in range(NIT):
                its = slice(it * 128, (it + 1) * 128)
                rd = 1 + it // 4            # strip region with the f2f diag
                start_col = 384 - 128 * (it % 4)
                for sname in ("f", "e"):
                    l_t = lp.tile([128, C], BF16, tag="l")
                    for half in range(2):
                        ps = psum.tile([128, 2048], F32, tag="ps")
                        for kp in range(2):
                            for nt in range(4):
                                rn = half * 4 + nt
                                reg = ps[:, nt * 512:(nt + 1) * 512]
                                is_d = sname == "f" and rn == rd
                                nc.tensor.matmul(
                                    reg,
                                    lhsT=lhs_sb[:, 2 * kp:2 * kp + 2, its],
                                    rhs=rhs_sb[sname][
                                        :, 2 * kp:2 * kp + 2,
                                        rn * 512:(rn + 1) * 512
                                    ],
                                    start=(kp == 0),
                                    stop=(kp == 1 and not is_d),
                                    perf_mode=DR,
                                )
                                if kp == 1 and is_d:
                                    nc.tensor.matmul(
                                        reg,
                                        lhsT=eye_sb,
                                        rhs=mA_sb[:, start_col:start_col + 512],
                                        start=False,
                                        stop=True,
                                    )
                        nc.scalar.activation(
                            out=l_t[:, half * 2048:(half + 1) * 2048],
                            in_=ps,
                            func=AF.Exp,
                            bias=bias_sb,
                            scale=exp_scale,
                            accum_out=stall[
                                :, DH[sname] + 2 * it + half:
                                DH[sname] + 2 * it + half + 1
                            ],
                        )

                    # row max of l_t via bf16 TT tree + short reduce
                    m1 = mtp.tile([128, 2048], BF16, tag="m1")
                    nc.vector.tensor_tensor(
                        out=m1, in0=l_t[:, :2048], in1=l_t[:, 2048:],
                        op=OP.max,
                    )
                    m2 = mtp.tile([128, 1024], BF16, tag="m2")
                    nc.vector.tensor_tensor(
                        out=m2, in0=m1[:, :1024], in1=m1[:, 1024:],
                        op=OP.max,
                    )
                    m3 = mtp.tile([128, 512], BF16, tag="m3")
                    nc.vector.tensor_tensor(
                        out=m3, in0=m2[:, :512], in1=m2[:, 512:],
                        op=OP.max,
                    )
                    nc.vector.tensor_reduce(
                        out=stall[:, FM[sname] + it:FM[sname] + it + 1],
                        in_=m3,
                        axis=AX.X,
                        op=OP.max,
                    )
                    # masked numerator over the positive-band window only
                    w0 = 128 * it + WOFF
                    ml_t = mlp.tile([128, WLEN], mybir.dt.float8e4, tag="ml")
                    nc.vector.scalar_tensor_tensor(
                        out=ml_t,
                        in0=labc_sb[:, w0:w0 + WLEN],
                        scalar=labs_sb[:, it:it + 1],
                        in1=l_t[:, w0:w0 + WLEN],
                        op0=OP.is_equal,
                        op1=OP.mult,
                        accum_out=stall[:, FN[sname] + it:FN[sname] + it + 1],
                    )
                if it == NIT // 2 - 1:
                    ship_stats(0, NIT // 2)
            ship_stats(NIT // 2, NIT)

    nc.finalize()
    return nc


_NC_CACHE = None


def _get_nc():
    global _NC_CACHE
    if _NC_CACHE is None:
        _NC_CACHE = build_nc()
    return _NC_CACHE


def _prep_inputs(embed_data, embed_label, label):
    import ml_dtypes

    xd = np.asarray(embed_data, dtype=np.float32)
    xl = np.asarray(embed_label, dtype=np.float32)
    lab = np.asarray(label).astype(np.int64)

    def norm(x):
        n = np.sqrt(np.sum(x.astype(np.float64) ** 2, axis=1, keepdims=True))
        n = np.maximum(n, EPS)
        return (x / n).astype(np.float32)

    # label-sorted row/col order (loss is row-order invariant)
    perm = np.argsort(lab, kind="stable")
    lab_s = lab[perm]
    xdq = (norm(xd)[perm] * FSCALE).astype(ml_dtypes.float8_e4m3)
    xlq = (norm(xl)[perm] * FSCALE).astype(ml_dtypes.float8_e4m3)
    xdT = np.ascontiguousarray(xdq.T)   # [D, N] fp8
    xlT = np.ascontiguousarray(xlq.T)
    labf = _label_encode(lab_s)         # [N] f32, bf16-exact values

    # positive-band coverage check: all same-label cols of row tile
    # (r, it) must fall inside the static 512-col window
    first = np.searchsorted(lab_s, lab_s, side="left")
    last = np.searchsorted(lab_s, lab_s, side="right") - 1
    rs = np.arange(0, N, 128)
    lo = first[rs]
    hi = last[rs + 127]
    wstart = rs - (COLSH - WOFF)   # global window start (mod N)
    if not (
        np.all((lo - wstart) % N < WLEN) and np.all((hi - wstart) % N < WLEN)
    ):
        raise RuntimeError("positive band exceeds STT window")

    def kblocks(mat_T, cols):
        # [D, ncols] -> [128, KS*ncols] with k-subtile blocks back to back
        a = mat_T[:, cols]
        return np.ascontiguousarray(
            a.reshape(KS, 128, len(cols)).transpose(1, 0, 2).reshape(128, -1)
        )

    eye = np.zeros((128, 128), dtype=ml_dtypes.float8_e4m3)
    np.fill_diagonal(eye, EYEV)

    in_maps = []
    for k in range(NCORES):
        r, c = k % RGRID, k // RGRID
        rows = np.arange(R * r, R * (r + 1))
        col0 = R * r - COLSH if c == 0 else R * r - COLSH + C
        cols = (col0 + np.arange(C)) % N

        labs_a = labf[rows].reshape(NIT, 128).T.copy()    # [128, NIT]
        if c == 0:
            labc_a = np.broadcast_to(
                labf[cols].astype(ml_dtypes.bfloat16), (128, C)
            ).copy()
        else:
            labc_a = np.full((128, C), -1.0, dtype=ml_dtypes.bfloat16)

        mAb = np.zeros((128, 896), dtype=ml_dtypes.float8_e4m3)
        if c == 0:
            mAb[np.arange(128), 384 + np.arange(128)] = NEGM

        in_maps.append({
            "lhs": kblocks(xdT, rows),
            "rhsD": kblocks(xdT, cols),
            "rhsL": kblocks(xlT, cols),
            "labs": np.ascontiguousarray(labs_a),
            "labc": labc_a,
            "mA": mAb,
            "eyeK": eye,
        })
    return in_maps


def _combine(results):
    """Host combine of per-core shard stats -> scalar loss (fp64)."""
    # ofe groups: df, mf, nf, de, me, ne at columns g*NIT
    GIDX = {"df": 0, "mf": 1, "nf": 2, "de": 3, "me": 4, "ne": 5}

    def get(name):
        g = GIDX[name]
        out = np.empty((RGRID, CGRID, 128, NIT), dtype=np.float64)
        for k in range(NCORES):
            r, c = k % RGRID, k // RGRID
            out[r, c] = results[k]["ofe"][
                :, g * NIT:(g + 1) * NIT
            ].astype(np.float64)
        return out

    nf, df, mf = get("nf"), get("df"), get("mf")
    ne, de, me = get("ne"), get("de"), get("me")

    # sorted row g = 2048 r + 128 it + p  <->  [r, c, p, it]
    # mf/me hold max_j l' = exp((Ms - SHIFT)/T); the reference weight
    # e^{(SHIFT - Ms)/T} is just its reciprocal.
    Mlf = np.max(mf, axis=1)           # [RGRID, 128, NIT]
    Mle = np.max(me, axis=1)
    Af = np.sum(nf, axis=1)
    Cf = np.sum(df, axis=1)
    Ae = np.sum(ne, axis=1)
    Ce = np.sum(de, axis=1)

    wf = 1.0 / Mlf
    we = 1.0 / Mle
    num = we * Ae + wf * Af
    den = we * Ce + wf * Cf
    row_loss = np.log(den) - np.log(num)
    return np.float32(np.mean(row_loss))


def kernel(embed_data, embed_label, label):
    nc = _get_nc()
    in_maps = _prep_inputs(embed_data, embed_label, label)
    res = run_bass_kernel_spmd(nc, in_maps, list(range(NCORES)))
    return _combine(res.results)


if __name__ == "__main__":
    rng = np.random.default_rng(0)
    ed = rng.standard_normal((N, D), dtype=np.float32)
    el = rng.standard_normal((N, D), dtype=np.float32)
    lb = rng.integers(0, 1000, N)
    print(kernel(ed, el, lb))
